# revision 20
# baseline (speedup 1.0000x reference)
"""Distributed GQA attention (RoPE + causal mask + o_proj) on 8 TRN2 NeuronCores.

Sharding: 8-way tensor parallel over heads. Core c handles q heads
[4c, 4c+4) and kv head c (the matching GQA group) for BOTH batches.
Per-core flow:
  xT (host-pre-transposed [D, B*S] bf16) feeds all projections
  qT/kT = W^T x^T via TensorE (head dims on partitions, seq on free axis);
  k and v share one packed projection matmul
  RoPE applied in the transposed layout (swap-halves via a PE permutation
  matmul; cos/sin tables prepared on host, q side pre-scaled by 1/sqrt(hd))
  scores^T[k, q] = kT.T @ qT per head, two heads packed into disjoint
  PE row-groups, two k-tiles per PSUM tile so exp runs on [128,1024]
  causal masking: skip fully-masked k-tiles, multiply the diagonal tiles
  by a 0/1 bf16 mask post-exp
  attn^T[dv, q] (+ a ones-row giving the softmax denominators) = v_aug @ probs
  normalize with reciprocal_approx_fast + gpsimd partition_broadcast
  one 8-core mesh AllToAll per batch re-shards from head-split to
  row-split (the batch-0 collective overlaps batch-1 attention); o_proj
  consumes attn^T chunks as stationary operands against full wo; core c
  emits rows [c*S/8, (c+1)*S/8) of batch 0 then of batch 1.
"""

import os
import sys

for _p in ("/opt/trn_rl_repo", "/root/.axon_site/_ro/trn_rl_repo"):
    if os.path.isdir(_p) and _p not in sys.path:
        sys.path.append(_p)

import numpy as np
import ml_dtypes

import concourse.bass as bass
import concourse.bacc as bacc
import concourse.tile as tile
import concourse.mybir as mybir
from concourse import bass_utils

FP32 = mybir.dt.float32
BF16 = mybir.dt.bfloat16
AF = mybir.ActivationFunctionType
ALU = mybir.AluOpType
PSUM = bass.MemorySpace.PSUM
NPBF16 = ml_dtypes.bfloat16

B = 2
D = 2048
S = 2048
HD = 64
N_HEADS = 32
N_KV = 8
NCORES = 8
HQ = N_HEADS // NCORES   # 4 local q heads
QCOLS = HQ * HD          # 256
NDC = D // 128           # 16 contraction chunks
NPAIR = HQ // 2          # 2 head pairs
THETA = 10000.0


def build_graph(causal: bool, s: int = S):
    """Build + compile the per-core SPMD graph. Identical on all 8 cores."""
    bs = B * s               # flattened rows
    nqb = s // 512           # q blocks per batch
    nkt = s // 128           # k tiles per batch
    nrs = bs // 512          # projection row slices (flattened)
    rows_h = s // NCORES     # output rows per core per batch
    rt_m = min(128, rows_h)  # o_proj row-tile height
    nrt_h = rows_h // rt_m   # o_proj row tiles per batch half

    nc = bacc.Bacc("TRN2", target_bir_lowering=False, debug=False,
                   enable_asserts=True, num_devices=NCORES)

    xT_h = nc.dram_tensor("xT", [D, bs], BF16, kind="ExternalInput")
    wq_h = nc.dram_tensor("wq", [D, QCOLS], BF16, kind="ExternalInput")
    wkv_h = nc.dram_tensor("wkv", [D, 2 * HD], BF16, kind="ExternalInput")
    wo_h = nc.dram_tensor("wo", [D, D], BF16, kind="ExternalInput")
    c4_h = nc.dram_tensor("c4", [128, s], FP32, kind="ExternalInput")
    s4_h = nc.dram_tensor("s4", [128, s], FP32, kind="ExternalInput")
    id_h = nc.dram_tensor("ident", [128, 128], BF16, kind="ExternalInput")
    pw_h = nc.dram_tensor("pswap", [128, 128], BF16, kind="ExternalInput")
    if causal:
        md_h = nc.dram_tensor("mdiag", [128, 1024], BF16, kind="ExternalInput")
    else:
        mT_h = nc.dram_tensor("maskT", [s, s], FP32, kind="ExternalInput")
    out_h = nc.dram_tensor("out", [B * rows_h, D], FP32, kind="ExternalOutput")

    with tile.TileContext(nc) as tc:
        with tc.tile_pool(name="persist", bufs=1) as pp, \
             tc.tile_pool(name="dram", bufs=1, space="DRAM") as dramp:

            # ---- constants ----
            c4 = pp.tile([128, s], FP32, tag="c4", name="c4t")
            s4 = pp.tile([128, s], FP32, tag="s4", name="s4t")
            nc.gpsimd.dma_start(c4[:, :], c4_h[:, :])
            nc.gpsimd.dma_start(s4[:, :], s4_h[:, :])
            ident = pp.tile([128, 128], BF16, tag="idb", name="identb")
            nc.gpsimd.dma_start(ident[:, :], id_h[:, :])
            psw = pp.tile([128, 128], BF16, tag="pwb", name="pswapb")
            nc.gpsimd.dma_start(psw[:, :], pw_h[:, :])
            if causal:
                md = pp.tile([128, 1024], BF16, tag="mdb", name="mdb")
                nc.gpsimd.dma_start(md[:, :], md_h[:, :])

            # ---- persistent activations ----
            qT = [pp.tile([128, bs], BF16, tag=f"qT{i}", name=f"qT{i}")
                  for i in range(NPAIR)]
            kTrep = [pp.tile([128, s], BF16, tag=f"kTr{i}", name=f"kTrep{i}")
                     for i in range(B)]
            vb = [[pp.tile([128, HD + 1], BF16, tag=f"vb{b}_{i}",
                           name=f"vb{b}_{i}") for i in range(nkt)]
                  for b in range(B)]
            # attn^T, one [64, bs] tile per local head (partition base 0)
            attnT = [pp.tile([64, bs], BF16, tag=f"aT{i}", name=f"attnT{i}")
                     for i in range(HQ)]

            bnc_in = [[dramp.tile([NCORES, 2 * HD, rows_h], BF16,
                                  tag=f"bin{b}_{hp}", name=f"bounce_in{b}_{hp}")
                       for hp in range(NPAIR)] for b in range(B)]
            bnc_out = [[dramp.tile([NCORES, 2 * HD, rows_h], BF16,
                                   tag=f"bout{b}_{hp}", name=f"bounce_out{b}_{hp}")
                        for hp in range(NPAIR)] for b in range(B)]

            # ================= phase B: projections + RoPE =================
            def rope(src_ps, swap_ps, scale, dst_ap, sl0, npart, rp, nm):
                """dst = (src * scale) .* c4 + (swap(src) * scale) .* s4."""
                cs = c4[0:npart, sl0:sl0 + 512]
                sn = s4[0:npart, sl0:sl0 + 512]
                m1 = rp.tile([npart, 512], FP32, tag=f"m1_{npart}", name=f"m1{nm}")
                nc.vector.scalar_tensor_tensor(m1[:, :], src_ps, scale, cs,
                                               ALU.mult, ALU.mult)
                m2 = rp.tile([npart, 512], FP32, tag=f"m2_{npart}", name=f"m2{nm}")
                nc.vector.scalar_tensor_tensor(m2[:, :], swap_ps, scale, sn,
                                               ALU.mult, ALU.mult)
                nc.vector.tensor_add(dst_ap, m1[:, :], m2[:, :])

            with tc.tile_pool(name="wres", bufs=1) as wrp, \
                 tc.tile_pool(name="xpool", bufs=24) as xp, \
                 tc.tile_pool(name="ropes", bufs=3) as rp, \
                 tc.tile_pool(name="qps_p", bufs=2, space=PSUM) as pq, \
                 tc.tile_pool(name="qsw_p", bufs=2, space=PSUM) as pqs, \
                 tc.tile_pool(name="kv_p", bufs=1, space=PSUM) as pkv, \
                 tc.tile_pool(name="tp_p", bufs=1, space=PSUM) as ptp:

                # batched weight loads: 4 D-chunks per DMA, issue split across
                # two HWDGE queues (startup is DMA-issue-serialized)
                wq_all = wrp.tile([128, NDC * QCOLS], BF16, tag="wqa",
                                  name="wq_all")
                wkv_all = wrp.tile([128, NDC * 2 * HD], BF16, tag="wkva",
                                   name="wkv_all")
                for g in range(4):
                    dst = wq_all[:, g * 4 * QCOLS:(g + 1) * 4 * QCOLS].rearrange(
                        "p (c q) -> p c q", c=4)
                    src = wq_h[g * 512:(g + 1) * 512, :].rearrange(
                        "(c p) q -> p c q", p=128)
                    nc.sync.dma_start(dst, src)
                    dstk = wkv_all[:, g * 8 * HD:(g + 1) * 8 * HD].rearrange(
                        "p (c q) -> p c q", c=4)
                    srck = wkv_h[g * 512:(g + 1) * 512, :].rearrange(
                        "(c p) q -> p c q", p=128)
                    nc.scalar.dma_start(dstk, srck)
                wqb = [wq_all[:, dc * QCOLS:(dc + 1) * QCOLS]
                       for dc in range(NDC)]
                wkvb = [wkv_all[:, dc * 2 * HD:(dc + 1) * 2 * HD]
                        for dc in range(NDC)]

                nxp = bs // 1024      # xT tiles, each covering 2 row slices
                for xi in range(nxp):
                    x0 = xi * 1024
                    xtb = []
                    for dc in range(NDC):
                        xt = xp.tile([128, 1024], BF16, tag="xtb",
                                     name=f"xtb{xi}_{dc}")
                        eng = nc.sync if dc % 2 == 0 else nc.scalar
                        eng.dma_start(
                            xt[:, :], xT_h[dc * 128:(dc + 1) * 128, x0:x0 + 1024])
                        xtb.append(xt)

                    for sub in range(2):
                        rs = xi * 2 + sub
                        r0 = rs * 512          # flattened row offset
                        b = r0 // s            # batch of this slice
                        sl0 = r0 - b * s       # seq offset within batch
                        xsl = slice(sub * 512, sub * 512 + 512)

                        # q projection + rope, one head pair at a time
                        for hp in range(NPAIR):
                            qps = pq.tile([128, 512], FP32, tag="qps",
                                          name=f"qps{rs}_{hp}")
                            for dc in range(NDC):
                                nc.tensor.matmul(qps[:, :],
                                                 wqb[dc][:, hp * 128:(hp + 1) * 128],
                                                 xtb[dc][:, xsl],
                                                 start=(dc == 0), stop=(dc == NDC - 1))
                            qsb = rp.tile([128, 512], BF16, tag="qsb",
                                          name=f"qsb{rs}_{hp}")
                            nc.scalar.copy(qsb[:, :], qps[:, :])
                            qsw = pqs.tile([128, 512], FP32, tag="qsw",
                                           name=f"qsw{rs}_{hp}")
                            nc.tensor.matmul(qsw[:, :], psw[:, :], qsb[:, :],
                                             start=True, stop=True)
                            rope(qps[:, :], qsw[:, :], 0.125,
                                 qT[hp][:, r0:r0 + 512], sl0, 128, rp, f"q{rs}_{hp}")

                        # packed k|v projection: psum rows 0:64 = kT, 64:128 = vT
                        kvps = pkv.tile([128, 512], FP32, tag="kvps",
                                        name=f"kvps{rs}")
                        for dc in range(NDC):
                            nc.tensor.matmul(kvps[:, :], wkvb[dc][:, :],
                                             xtb[dc][:, xsl],
                                             start=(dc == 0), stop=(dc == NDC - 1))
                        # k: rope + replicate into both 64-partition halves
                        ksb = rp.tile([64, 512], BF16, tag="ksb", name=f"ksb{rs}")
                        nc.scalar.copy(ksb[:, :], kvps[0:64, :])
                        ksw = pqs.tile([64, 512], FP32, tag="qsw", name=f"ksw{rs}")
                        nc.tensor.matmul(ksw[:, :], psw[0:64, 0:64], ksb[:, :],
                                         start=True, stop=True)
                        ktmp = rp.tile([64, 512], BF16, tag="ktmp", name=f"ktmp{rs}")
                        rope(kvps[0:64, :], ksw[:, :], 1.0, ktmp[:, :], sl0, 64,
                             rp, f"k{rs}")
                        nc.sync.dma_start(kTrep[b][0:64, sl0:sl0 + 512], ktmp[:, :])
                        nc.sync.dma_start(kTrep[b][64:128, sl0:sl0 + 512], ktmp[:, :])
                        # v: PE-transpose vT to row-major + ones col
                        vts = rp.tile([128, 512], BF16, tag="vts", name=f"vts{rs}")
                        nc.vector.tensor_copy(vts[64:128, :], kvps[64:128, :])
                        for rb in range(4):
                            kt = sl0 // 128 + rb
                            tps = ptp.tile([128, HD], BF16, tag="tps",
                                           name=f"vt{rs}_{rb}")
                            nc.tensor.transpose(tps[:, :],
                                                vts[64:128, rb * 128:(rb + 1) * 128],
                                                ident[64:128, 64:128])
                            nc.vector.tensor_copy(vb[b][kt][:, 0:HD], tps[:, :])
                            nc.vector.memset(vb[b][kt][:, HD:HD + 1], 1.0)

            # wo tiles prefetch during attention (no deps besides SBUF slots)
            wop_ctx = tc.tile_pool(name="wob", bufs=1)
            wop = wop_ctx.__enter__()
            wot = []
            for ch in range(NDC):
                wt = wop.tile([128, D], BF16, tag=f"wob{ch}", name=f"wob{ch}")
                nc.sync.dma_start(wt[:, :], wo_h[ch * 128:(ch + 1) * 128, :])
                wot.append(wt)

            # ================= attention (+ per-batch A2A) =================
            with tc.tile_pool(name="sc_p", bufs=1, space=PSUM) as psc, \
                 tc.tile_pool(name="at_p", bufs=2, space=PSUM) as pat, \
                 tc.tile_pool(name="probs", bufs=4) as prp, \
                 tc.tile_pool(name="maskq", bufs=nkt) as mqp, \
                 tc.tile_pool(name="norm", bufs=4) as nrm:

                for b in range(B):
                    for hp in range(NPAIR):
                        for qb in range(nqb):
                            q0 = qb * 512            # seq offset within batch
                            g0 = b * s + q0          # flattened offset
                            kt_end = 4 * (qb + 1) if causal else nkt
                            mts = []
                            if not causal:
                                for kt in range(nkt):
                                    mt = mqp.tile([128, 512], FP32, tag="mq",
                                                  name=f"mq{b}_{hp}_{qb}_{kt}")
                                    nc.sync.dma_start(
                                        mt[:, :],
                                        mT_h[kt * 128:(kt + 1) * 128, q0:q0 + 512])
                                    mts.append(mt)
                            acc = [pat.tile([HD + 1, 512], FP32, tag=f"a{par}",
                                            name=f"a{par}_{b}_{qb}_{hp}")
                                   for par in range(2)]
                            for kp in range(kt_end // 2):
                                sc = psc.tile([128, 2048], FP32, tag="sc",
                                              name=f"sc_{b}_{qb}_{hp}_{kp}")
                                for par in range(2):
                                    pr = par * 64
                                    for i in range(2):
                                        kt = 2 * kp + i
                                        k0 = kt * 128
                                        o0 = par * 1024 + i * 512
                                        nc.tensor.matmul(
                                            sc[:, o0:o0 + 512],
                                            kTrep[b][pr:pr + 64, k0:k0 + 128],
                                            qT[hp][pr:pr + 64, g0:g0 + 512],
                                            start=True, stop=True)
                                        if not causal:
                                            nc.vector.tensor_add(
                                                sc[:, o0:o0 + 512],
                                                sc[:, o0:o0 + 512],
                                                mts[kt][:, :])
                                pb = prp.tile([128, 2048], BF16, tag="pb",
                                              name=f"pb_{b}_{qb}_{hp}_{kp}")
                                nc.scalar.activation(pb[:, :], sc[:, :], AF.Exp)
                                for par in range(2):
                                    for i in range(2):
                                        kt = 2 * kp + i
                                        o0 = par * 1024 + i * 512
                                        if causal and kt >= 4 * qb:
                                            delta = kt * 128 - q0
                                            msl = md[:, 512 - delta:1024 - delta]
                                            nc.vector.tensor_mul(
                                                pb[:, o0:o0 + 512],
                                                pb[:, o0:o0 + 512], msl)
                                        nc.tensor.matmul(
                                            acc[par][:, :],
                                            vb[b][kt][:, :],
                                            pb[:, o0:o0 + 512],
                                            start=(kt == 0),
                                            stop=(kt == kt_end - 1))
                            for par in range(2):
                                head = hp * 2 + par
                                # quick-release the PSUM bank: one copy to
                                # SBUF, the normalization chain reads that
                                a_sb = nrm.tile([HD + 1, 512], FP32, tag="asb",
                                                name=f"asb{b}_{qb}_{hp}_{par}")
                                nc.vector.tensor_copy(a_sb[:, :], acc[par][:, :])
                                rc = nrm.tile([1, 512], FP32, tag="rc",
                                              name=f"rc{b}_{qb}_{hp}_{par}")
                                nc.sync.dma_start(rc[0:1, :], a_sb[HD:HD + 1, :])
                                rcr = nrm.tile([1, 512], FP32, tag="rcr",
                                               name=f"rcr{b}_{qb}_{hp}_{par}")
                                nc.vector.reciprocal_approx_fast(rcr[0:1, :],
                                                                 rc[0:1, :])
                                rcb = nrm.tile([64, 512], FP32, tag="rcb",
                                               name=f"rcb{b}_{qb}_{hp}_{par}")
                                nc.gpsimd.partition_broadcast(rcb[:, :],
                                                              rcr[0:1, :])
                                nc.vector.tensor_mul(attnT[head][:, g0:g0 + 512],
                                                     a_sb[0:HD, :], rcb[:, :])

                        # re-shard this head pair (early pairs' collectives
                        # overlap the remaining attention compute)
                        for j in range(NCORES):
                            for par in range(2):
                                head = hp * 2 + par
                                nc.sync.dma_start(
                                    bnc_in[b][hp][j, par * 64:(par + 1) * 64, :],
                                    attnT[head][:, b * s + j * rows_h:
                                                b * s + (j + 1) * rows_h])
                        nc.gpsimd.collective_compute(
                            "AllToAll", ALU.bypass,
                            replica_groups=[list(range(NCORES))],
                            ins=[bnc_in[b][hp].opt()],
                            outs=[bnc_out[b][hp].opt()],
                        )

            # ================= o_proj =================
            with tc.tile_pool(name="att2", bufs=1) as a2p, \
                 tc.tile_pool(name="yout", bufs=3) as yop, \
                 tc.tile_pool(name="y_p", bufs=2, space=PSUM) as pyo:

                for b in range(B):
                    att2 = []
                    for i in range(NCORES):
                        for hp in range(NPAIR):
                            t = a2p.tile([128, rows_h], BF16,
                                         tag=f"at2_{b}_{i}_{hp}",
                                         name=f"att2_{b}_{i}_{hp}")
                            nc.sync.dma_start(t[:, :], bnc_out[b][hp][i, :, :])
                            att2.append(t)
                    for oc in range(D // 512):
                        o0 = oc * 512
                        for rt in range(nrt_h):
                            yps = pyo.tile([rt_m, 512], FP32, tag="y",
                                           name=f"y{b}_{oc}_{rt}")
                            for ch in range(NDC):
                                nc.tensor.matmul(
                                    yps[:, :],
                                    att2[ch][:, rt * rt_m:(rt + 1) * rt_m],
                                    wot[ch][:, o0:o0 + 512],
                                    start=(ch == 0), stop=(ch == NDC - 1))
                            ysb = yop.tile([rt_m, 512], FP32, tag="ysb",
                                           name=f"ysb{b}_{oc}_{rt}")
                            nc.vector.tensor_copy(ysb[:, :], yps[:, :])
                            nc.sync.dma_start(
                                out_h[b * rows_h + rt * rt_m:
                                      b * rows_h + (rt + 1) * rt_m,
                                      o0:o0 + 512],
                                ysb[:, :])

            wop_ctx.__exit__(None, None, None)

    nc.compile()
    return nc


# ===================== host side =====================

def _rope_tables(s):
    freqs = THETA ** (-np.arange(0, HD, 2, dtype=np.float64) / HD)   # [32]
    ang = np.arange(s, dtype=np.float64)[:, None] * freqs[None, :]   # [s, 32]
    cosT = np.cos(ang).T.astype(np.float32)                          # [32, s]
    sinT = np.sin(ang).T.astype(np.float32)
    c4 = np.tile(cosT, (4, 1))                                       # [128, s]
    s4 = np.tile(np.concatenate([-sinT, sinT], axis=0), (2, 1))      # [128, s]
    return np.ascontiguousarray(c4), np.ascontiguousarray(s4)


def _pswap():
    # permutation matrix: swap 32-halves within each 64 block (symmetric)
    p = np.zeros((128, 128), dtype=np.float32)
    for blk in range(2):
        for i in range(32):
            p[blk * 64 + i, blk * 64 + 32 + i] = 1.0
            p[blk * 64 + 32 + i, blk * 64 + i] = 1.0
    return p


def _mdiag():
    # keep[p, u] = 1 iff u >= p + 512 (sliced per diagonal tile offset)
    u = np.arange(1024)[None, :]
    p = np.arange(128)[:, None]
    return (u >= p + 512).astype(np.float32)


def _perm_even_odd(w, n_heads_w):
    # reorder each head's 64 columns: even indices first, then odd
    perm = np.concatenate([np.arange(0, HD, 2), np.arange(1, HD, 2)])
    wr = w.reshape(D, n_heads_w, HD)[:, :, perm]
    return np.ascontiguousarray(wr.reshape(D, n_heads_w * HD))


def _is_causal(mask, s):
    m = np.asarray(mask, dtype=np.float32).reshape(s, s)
    tri = np.tril(np.ones((s, s), dtype=bool))
    return bool(np.all(m[tri] == 0.0) and np.all(m[~tri] <= -1e8))


def _bf16(a):
    return np.ascontiguousarray(np.asarray(a, np.float32).astype(NPBF16))


def make_in_maps(x, mask, wq, wk, wv, wo, s=S):
    """Shard full inputs into 8 per-core input dicts."""
    causal = _is_causal(mask, s)
    c4, s4 = _rope_tables(s)
    wq_p = _perm_even_odd(np.asarray(wq, np.float32), N_HEADS)
    wk_p = _perm_even_odd(np.asarray(wk, np.float32), N_KV)
    wv = np.asarray(wv, np.float32)
    wo_b = _bf16(wo)
    ident = np.eye(128, dtype=np.float32).astype(NPBF16)
    psw = _pswap().astype(NPBF16)
    md = _mdiag().astype(NPBF16)
    xT = _bf16(np.asarray(x, np.float32).reshape(B * s, D).T)
    mT = None
    if not causal:
        mT = np.ascontiguousarray(np.asarray(mask, np.float32).reshape(s, s).T)

    in_maps = []
    for c in range(NCORES):
        wkv = np.concatenate([wk_p[:, c * HD:(c + 1) * HD],
                              wv[:, c * HD:(c + 1) * HD]], axis=1)
        im = {
            "xT": xT,
            "wq": _bf16(wq_p[:, c * QCOLS:(c + 1) * QCOLS]),
            "wkv": _bf16(wkv),
            "wo": wo_b,
            "c4": c4,
            "s4": s4,
            "ident": ident,
            "pswap": psw,
        }
        if causal:
            im["mdiag"] = md
        else:
            im["maskT"] = mT
        in_maps.append(im)
    return causal, in_maps


def assemble_output(per_core_outs, s=S):
    rows_h = s // NCORES
    y = np.empty((B, s, D), dtype=np.float32)
    for c in range(NCORES):
        o = np.asarray(per_core_outs[c], np.float32)
        for b in range(B):
            y[b, c * rows_h:(c + 1) * rows_h, :] = \
                o[b * rows_h:(b + 1) * rows_h]
    return y


_GRAPH_CACHE = {}


def get_graph(causal, s=S):
    key = (causal, s)
    if key not in _GRAPH_CACHE:
        _GRAPH_CACHE[key] = build_graph(causal, s)
    return _GRAPH_CACHE[key]


def kernel(**inputs):
    x = np.asarray(inputs["x"], np.float32)
    mask = inputs["mask"]
    s = x.shape[1]
    causal, in_maps = make_in_maps(x, mask, inputs["wq"], inputs["wk"],
                                   inputs["wv"], inputs["wo"], s=s)
    nc = get_graph(causal, s)
    res = bass_utils.run_bass_kernel_spmd(nc, in_maps, core_ids=list(range(NCORES)))
    return assemble_output([res.results[c]["out"] for c in range(NCORES)], s=s)


# revision 21
# speedup vs baseline: 1.4554x; 1.4554x over previous
"""Distributed GQA attention (RoPE + causal mask + o_proj) on 8 TRN2 NeuronCores.

Sharding: 8-way tensor parallel over heads. Core c handles q heads
[4c, 4c+4) and kv head c (the matching GQA group) for BOTH batches.
Per-core flow:
  xT (host-pre-transposed [D, B*S] bf16) feeds all projections
  qT/kT = W^T x^T via TensorE (head dims on partitions, seq on free axis);
  k and v share one packed projection matmul
  RoPE applied in the transposed layout (swap-halves via a PE permutation
  matmul; cos/sin tables prepared on host, q side pre-scaled by 1/sqrt(hd))
  scores^T[k, q] = kT.T @ qT per head, two heads packed into disjoint
  PE row-groups, two k-tiles per PSUM tile so exp runs on [128,1024]
  causal masking: skip fully-masked k-tiles, multiply the diagonal tiles
  by a 0/1 bf16 mask post-exp
  attn^T[dv, q] (+ a ones-row giving the softmax denominators) = v_aug @ probs
  normalize with reciprocal_approx_fast + gpsimd partition_broadcast
  one 8-core mesh AllToAll per batch re-shards from head-split to
  row-split (the batch-0 collective overlaps batch-1 attention); o_proj
  consumes attn^T chunks as stationary operands against full wo; core c
  emits rows [c*S/8, (c+1)*S/8) of batch 0 then of batch 1.
"""

import os
import sys

for _p in ("/opt/trn_rl_repo", "/root/.axon_site/_ro/trn_rl_repo"):
    if os.path.isdir(_p) and _p not in sys.path:
        sys.path.append(_p)

import numpy as np
import ml_dtypes

import concourse.bass as bass
import concourse.bacc as bacc
import concourse.tile as tile
import concourse.mybir as mybir
from concourse import bass_utils

FP32 = mybir.dt.float32
BF16 = mybir.dt.bfloat16
AF = mybir.ActivationFunctionType
ALU = mybir.AluOpType
PSUM = bass.MemorySpace.PSUM
NPBF16 = ml_dtypes.bfloat16

B = 2
D = 2048
S = 2048
HD = 64
N_HEADS = 32
N_KV = 8
NCORES = 8
HQ = N_HEADS // NCORES   # 4 local q heads
QCOLS = HQ * HD          # 256
NDC = D // 128           # 16 contraction chunks
NPAIR = HQ // 2          # 2 head pairs
THETA = 10000.0


def build_graph(causal: bool, s: int = S):
    """Build + compile the per-core SPMD graph. Identical on all 8 cores."""
    bs = B * s               # flattened rows
    nqb = s // 512           # q blocks per batch
    nkt = s // 128           # k tiles per batch
    nrs = bs // 512          # projection row slices (flattened)
    rows_h = s // NCORES     # output rows per core per batch
    rt_m = min(128, rows_h)  # o_proj row-tile height
    nrt_h = rows_h // rt_m   # o_proj row tiles per batch half

    nc = bacc.Bacc("TRN2", target_bir_lowering=False, debug=False,
                   enable_asserts=True, num_devices=NCORES)

    xT_h = nc.dram_tensor("xT", [D, bs], BF16, kind="ExternalInput")
    wq_h = nc.dram_tensor("wq", [D, QCOLS], BF16, kind="ExternalInput")
    wkv_h = nc.dram_tensor("wkv", [D, 2 * HD], BF16, kind="ExternalInput")
    wo_h = nc.dram_tensor("wo", [D, D], BF16, kind="ExternalInput")
    c4_h = nc.dram_tensor("c4", [128, s], FP32, kind="ExternalInput")
    s4_h = nc.dram_tensor("s4", [128, s], FP32, kind="ExternalInput")
    id_h = nc.dram_tensor("ident", [128, 128], BF16, kind="ExternalInput")
    pw_h = nc.dram_tensor("pswap", [128, 128], BF16, kind="ExternalInput")
    if causal:
        md_h = nc.dram_tensor("mdiag", [128, 1024], BF16, kind="ExternalInput")
    else:
        mT_h = nc.dram_tensor("maskT", [s, s], FP32, kind="ExternalInput")
    out_h = nc.dram_tensor("out", [B * rows_h, D], FP32, kind="ExternalOutput")

    with tile.TileContext(nc) as tc:
        with tc.tile_pool(name="persist", bufs=1) as pp, \
             tc.tile_pool(name="dram", bufs=1, space="DRAM") as dramp:

            # ---- constants ----
            c4 = pp.tile([128, s], FP32, tag="c4", name="c4t")
            s4 = pp.tile([128, s], FP32, tag="s4", name="s4t")
            nc.gpsimd.dma_start(c4[:, :], c4_h[:, :])
            nc.gpsimd.dma_start(s4[:, :], s4_h[:, :])
            ident = pp.tile([128, 128], BF16, tag="idb", name="identb")
            nc.gpsimd.dma_start(ident[:, :], id_h[:, :])
            psw = pp.tile([128, 128], BF16, tag="pwb", name="pswapb")
            nc.gpsimd.dma_start(psw[:, :], pw_h[:, :])
            if causal:
                md = pp.tile([128, 1024], BF16, tag="mdb", name="mdb")
                nc.gpsimd.dma_start(md[:, :], md_h[:, :])

            # ---- persistent activations ----
            qT = [pp.tile([128, bs], BF16, tag=f"qT{i}", name=f"qT{i}")
                  for i in range(NPAIR)]
            kTrep = [pp.tile([128, s], BF16, tag=f"kTr{i}", name=f"kTrep{i}")
                     for i in range(B)]
            vb = [[pp.tile([128, HD + 1], BF16, tag=f"vb{b}_{i}",
                           name=f"vb{b}_{i}") for i in range(nkt)]
                  for b in range(B)]
            # attn^T, one [64, bs] tile per local head (partition base 0)
            attnT = [pp.tile([64, bs], BF16, tag=f"aT{i}", name=f"attnT{i}")
                     for i in range(HQ)]

            bnc_in = [[dramp.tile([NCORES, 2 * HD, rows_h], BF16,
                                  tag=f"bin{b}_{hp}", name=f"bounce_in{b}_{hp}")
                       for hp in range(NPAIR)] for b in range(B)]
            bnc_out = [[dramp.tile([NCORES, 2 * HD, rows_h], BF16,
                                   tag=f"bout{b}_{hp}", name=f"bounce_out{b}_{hp}")
                        for hp in range(NPAIR)] for b in range(B)]

            # ================= phase B: projections + RoPE =================
            def rope(src_ps, swap_ps, scale, dst_ap, sl0, npart, rp, nm):
                """dst = (src * scale) .* c4 + (swap(src) * scale) .* s4."""
                cs = c4[0:npart, sl0:sl0 + 512]
                sn = s4[0:npart, sl0:sl0 + 512]
                m1 = rp.tile([npart, 512], FP32, tag=f"m1_{npart}", name=f"m1{nm}")
                nc.vector.scalar_tensor_tensor(m1[:, :], src_ps, scale, cs,
                                               ALU.mult, ALU.mult)
                m2 = rp.tile([npart, 512], FP32, tag=f"m2_{npart}", name=f"m2{nm}")
                nc.vector.scalar_tensor_tensor(m2[:, :], swap_ps, scale, sn,
                                               ALU.mult, ALU.mult)
                nc.vector.tensor_add(dst_ap, m1[:, :], m2[:, :])

            with tc.tile_pool(name="wres", bufs=1) as wrp, \
                 tc.tile_pool(name="xpool", bufs=24) as xp, \
                 tc.tile_pool(name="ropes", bufs=3) as rp, \
                 tc.tile_pool(name="qps_p", bufs=2, space=PSUM) as pq, \
                 tc.tile_pool(name="qsw_p", bufs=2, space=PSUM) as pqs, \
                 tc.tile_pool(name="kv_p", bufs=1, space=PSUM) as pkv, \
                 tc.tile_pool(name="tp_p", bufs=1, space=PSUM) as ptp:

                # batched weight loads: 4 D-chunks per DMA, issue split across
                # two HWDGE queues (startup is DMA-issue-serialized)
                wq_all = wrp.tile([128, NDC * QCOLS], BF16, tag="wqa",
                                  name="wq_all")
                wkv_all = wrp.tile([128, NDC * 2 * HD], BF16, tag="wkva",
                                   name="wkv_all")
                for g in range(4):
                    dst = wq_all[:, g * 4 * QCOLS:(g + 1) * 4 * QCOLS].rearrange(
                        "p (c q) -> p c q", c=4)
                    src = wq_h[g * 512:(g + 1) * 512, :].rearrange(
                        "(c p) q -> p c q", p=128)
                    nc.sync.dma_start(dst, src)
                    dstk = wkv_all[:, g * 8 * HD:(g + 1) * 8 * HD].rearrange(
                        "p (c q) -> p c q", c=4)
                    srck = wkv_h[g * 512:(g + 1) * 512, :].rearrange(
                        "(c p) q -> p c q", p=128)
                    nc.scalar.dma_start(dstk, srck)
                wqb = [wq_all[:, dc * QCOLS:(dc + 1) * QCOLS]
                       for dc in range(NDC)]
                wkvb = [wkv_all[:, dc * 2 * HD:(dc + 1) * 2 * HD]
                        for dc in range(NDC)]

                nxp = bs // 1024      # xT tiles, each covering 2 row slices
                for xi in range(nxp):
                    x0 = xi * 1024
                    xtb = []
                    for dc in range(NDC):
                        xt = xp.tile([128, 1024], BF16, tag="xtb",
                                     name=f"xtb{xi}_{dc}")
                        eng = nc.sync if dc % 2 == 0 else nc.scalar
                        eng.dma_start(
                            xt[:, :], xT_h[dc * 128:(dc + 1) * 128, x0:x0 + 1024])
                        xtb.append(xt)

                    for sub in range(2):
                        rs = xi * 2 + sub
                        r0 = rs * 512          # flattened row offset
                        b = r0 // s            # batch of this slice
                        sl0 = r0 - b * s       # seq offset within batch
                        xsl = slice(sub * 512, sub * 512 + 512)

                        # q projection + rope, one head pair at a time
                        for hp in range(NPAIR):
                            qps = pq.tile([128, 512], FP32, tag="qps",
                                          name=f"qps{rs}_{hp}")
                            for dc in range(NDC):
                                nc.tensor.matmul(qps[:, :],
                                                 wqb[dc][:, hp * 128:(hp + 1) * 128],
                                                 xtb[dc][:, xsl],
                                                 start=(dc == 0), stop=(dc == NDC - 1))
                            qsb = rp.tile([128, 512], BF16, tag="qsb",
                                          name=f"qsb{rs}_{hp}")
                            nc.scalar.copy(qsb[:, :], qps[:, :])
                            qsw = pqs.tile([128, 512], FP32, tag="qsw",
                                           name=f"qsw{rs}_{hp}")
                            nc.tensor.matmul(qsw[:, :], psw[:, :], qsb[:, :],
                                             start=True, stop=True)
                            rope(qps[:, :], qsw[:, :], 0.125,
                                 qT[hp][:, r0:r0 + 512], sl0, 128, rp, f"q{rs}_{hp}")

                        # packed k|v projection: psum rows 0:64 = kT, 64:128 = vT
                        kvps = pkv.tile([128, 512], FP32, tag="kvps",
                                        name=f"kvps{rs}")
                        for dc in range(NDC):
                            nc.tensor.matmul(kvps[:, :], wkvb[dc][:, :],
                                             xtb[dc][:, xsl],
                                             start=(dc == 0), stop=(dc == NDC - 1))
                        # k: rope + replicate into both 64-partition halves
                        ksb = rp.tile([64, 512], BF16, tag="ksb", name=f"ksb{rs}")
                        nc.scalar.copy(ksb[:, :], kvps[0:64, :])
                        ksw = pqs.tile([64, 512], FP32, tag="qsw", name=f"ksw{rs}")
                        nc.tensor.matmul(ksw[:, :], psw[0:64, 0:64], ksb[:, :],
                                         start=True, stop=True)
                        ktmp = rp.tile([64, 512], BF16, tag="ktmp", name=f"ktmp{rs}")
                        rope(kvps[0:64, :], ksw[:, :], 1.0, ktmp[:, :], sl0, 64,
                             rp, f"k{rs}")
                        nc.sync.dma_start(kTrep[b][0:64, sl0:sl0 + 512], ktmp[:, :])
                        nc.sync.dma_start(kTrep[b][64:128, sl0:sl0 + 512], ktmp[:, :])
                        # v: PE-transpose vT to row-major + ones col
                        vts = rp.tile([128, 512], BF16, tag="vts", name=f"vts{rs}")
                        nc.vector.tensor_copy(vts[64:128, :], kvps[64:128, :])
                        for rb in range(4):
                            kt = sl0 // 128 + rb
                            tps = ptp.tile([128, HD], BF16, tag="tps",
                                           name=f"vt{rs}_{rb}")
                            nc.tensor.transpose(tps[:, :],
                                                vts[64:128, rb * 128:(rb + 1) * 128],
                                                ident[64:128, 64:128])
                            nc.vector.tensor_copy(vb[b][kt][:, 0:HD], tps[:, :])
                            nc.vector.memset(vb[b][kt][:, HD:HD + 1], 1.0)

            # wo tiles prefetch during attention (no deps besides SBUF slots)
            wop_ctx = tc.tile_pool(name="wob", bufs=1)
            wop = wop_ctx.__enter__()
            wot = []
            for ch in range(NDC):
                wt = wop.tile([128, D], BF16, tag=f"wob{ch}", name=f"wob{ch}")
                nc.sync.dma_start(wt[:, :], wo_h[ch * 128:(ch + 1) * 128, :])
                wot.append(wt)

            # ================= attention (+ per-batch A2A) =================
            with tc.tile_pool(name="sc_p", bufs=1, space=PSUM) as psc, \
                 tc.tile_pool(name="at_p", bufs=2, space=PSUM) as pat, \
                 tc.tile_pool(name="probs", bufs=8) as prp, \
                 tc.tile_pool(name="maskq", bufs=nkt) as mqp, \
                 tc.tile_pool(name="norm", bufs=4) as nrm:

                for b in range(B):
                    for hp in range(NPAIR):
                        for qb in range(nqb):
                            q0 = qb * 512            # seq offset within batch
                            g0 = b * s + q0          # flattened offset
                            kt_end = 4 * (qb + 1) if causal else nkt
                            mts = []
                            if not causal:
                                for kt in range(nkt):
                                    mt = mqp.tile([128, 512], FP32, tag="mq",
                                                  name=f"mq{b}_{hp}_{qb}_{kt}")
                                    nc.sync.dma_start(
                                        mt[:, :],
                                        mT_h[kt * 128:(kt + 1) * 128, q0:q0 + 512])
                                    mts.append(mt)
                            acc = [pat.tile([HD + 1, 512], FP32, tag=f"a{par}",
                                            name=f"a{par}_{b}_{qb}_{hp}")
                                   for par in range(2)]
                            for kp in range(kt_end // 2):
                                for par in range(2):
                                    pr = par * 64
                                    sc = psc.tile([128, 1024], FP32, tag=f"sc{par}",
                                                  name=f"sc{par}_{b}_{qb}_{hp}_{kp}")
                                    for i in range(2):
                                        kt = 2 * kp + i
                                        k0 = kt * 128
                                        nc.tensor.matmul(
                                            sc[:, i * 512:(i + 1) * 512],
                                            kTrep[b][pr:pr + 64, k0:k0 + 128],
                                            qT[hp][pr:pr + 64, g0:g0 + 512],
                                            start=True, stop=True)
                                        if not causal:
                                            nc.vector.tensor_add(
                                                sc[:, i * 512:(i + 1) * 512],
                                                sc[:, i * 512:(i + 1) * 512],
                                                mts[kt][:, :])
                                    pb = prp.tile([128, 1024], BF16, tag=f"p{par}",
                                                  name=f"p{par}_{b}_{qb}_{hp}_{kp}")
                                    nc.scalar.activation(pb[:, :], sc[:, :], AF.Exp)
                                    for i in range(2):
                                        kt = 2 * kp + i
                                        if causal and kt >= 4 * qb:
                                            delta = kt * 128 - q0
                                            msl = md[:, 512 - delta:1024 - delta]
                                            nc.vector.tensor_mul(
                                                pb[:, i * 512:(i + 1) * 512],
                                                pb[:, i * 512:(i + 1) * 512], msl)
                                        nc.tensor.matmul(
                                            acc[par][:, :],
                                            vb[b][kt][:, :],
                                            pb[:, i * 512:(i + 1) * 512],
                                            start=(kt == 0),
                                            stop=(kt == kt_end - 1))
                            for par in range(2):
                                head = hp * 2 + par
                                # quick-release the PSUM bank: one copy to
                                # SBUF, the normalization chain reads that
                                a_sb = nrm.tile([HD + 1, 512], FP32, tag="asb",
                                                name=f"asb{b}_{qb}_{hp}_{par}")
                                nc.vector.tensor_copy(a_sb[:, :], acc[par][:, :])
                                rc = nrm.tile([1, 512], FP32, tag="rc",
                                              name=f"rc{b}_{qb}_{hp}_{par}")
                                nc.sync.dma_start(rc[0:1, :], a_sb[HD:HD + 1, :])
                                rcr = nrm.tile([1, 512], FP32, tag="rcr",
                                               name=f"rcr{b}_{qb}_{hp}_{par}")
                                nc.vector.reciprocal_approx_fast(rcr[0:1, :],
                                                                 rc[0:1, :])
                                rcb = nrm.tile([64, 512], FP32, tag="rcb",
                                               name=f"rcb{b}_{qb}_{hp}_{par}")
                                nc.gpsimd.partition_broadcast(rcb[:, :],
                                                              rcr[0:1, :])
                                nc.vector.tensor_mul(attnT[head][:, g0:g0 + 512],
                                                     a_sb[0:HD, :], rcb[:, :])

                        # re-shard this head pair (early pairs' collectives
                        # overlap the remaining attention compute)
                        for j in range(NCORES):
                            for par in range(2):
                                head = hp * 2 + par
                                nc.sync.dma_start(
                                    bnc_in[b][hp][j, par * 64:(par + 1) * 64, :],
                                    attnT[head][:, b * s + j * rows_h:
                                                b * s + (j + 1) * rows_h])
                        nc.gpsimd.collective_compute(
                            "AllToAll", ALU.bypass,
                            replica_groups=[list(range(NCORES))],
                            ins=[bnc_in[b][hp].opt()],
                            outs=[bnc_out[b][hp].opt()],
                        )

            # ================= o_proj =================
            with tc.tile_pool(name="att2", bufs=1) as a2p, \
                 tc.tile_pool(name="yout", bufs=3) as yop, \
                 tc.tile_pool(name="y_p", bufs=2, space=PSUM) as pyo:

                for b in range(B):
                    att2 = []
                    for i in range(NCORES):
                        for hp in range(NPAIR):
                            t = a2p.tile([128, rows_h], BF16,
                                         tag=f"at2_{b}_{i}_{hp}",
                                         name=f"att2_{b}_{i}_{hp}")
                            nc.sync.dma_start(t[:, :], bnc_out[b][hp][i, :, :])
                            att2.append(t)
                    for oc in range(D // 512):
                        o0 = oc * 512
                        for rt in range(nrt_h):
                            yps = pyo.tile([rt_m, 512], FP32, tag="y",
                                           name=f"y{b}_{oc}_{rt}")
                            for ch in range(NDC):
                                nc.tensor.matmul(
                                    yps[:, :],
                                    att2[ch][:, rt * rt_m:(rt + 1) * rt_m],
                                    wot[ch][:, o0:o0 + 512],
                                    start=(ch == 0), stop=(ch == NDC - 1))
                            ysb = yop.tile([rt_m, 512], FP32, tag="ysb",
                                           name=f"ysb{b}_{oc}_{rt}")
                            nc.vector.tensor_copy(ysb[:, :], yps[:, :])
                            nc.sync.dma_start(
                                out_h[b * rows_h + rt * rt_m:
                                      b * rows_h + (rt + 1) * rt_m,
                                      o0:o0 + 512],
                                ysb[:, :])

            wop_ctx.__exit__(None, None, None)

    nc.compile()
    return nc


# ===================== host side =====================

def _rope_tables(s):
    freqs = THETA ** (-np.arange(0, HD, 2, dtype=np.float64) / HD)   # [32]
    ang = np.arange(s, dtype=np.float64)[:, None] * freqs[None, :]   # [s, 32]
    cosT = np.cos(ang).T.astype(np.float32)                          # [32, s]
    sinT = np.sin(ang).T.astype(np.float32)
    c4 = np.tile(cosT, (4, 1))                                       # [128, s]
    s4 = np.tile(np.concatenate([-sinT, sinT], axis=0), (2, 1))      # [128, s]
    return np.ascontiguousarray(c4), np.ascontiguousarray(s4)


def _pswap():
    # permutation matrix: swap 32-halves within each 64 block (symmetric)
    p = np.zeros((128, 128), dtype=np.float32)
    for blk in range(2):
        for i in range(32):
            p[blk * 64 + i, blk * 64 + 32 + i] = 1.0
            p[blk * 64 + 32 + i, blk * 64 + i] = 1.0
    return p


def _mdiag():
    # keep[p, u] = 1 iff u >= p + 512 (sliced per diagonal tile offset)
    u = np.arange(1024)[None, :]
    p = np.arange(128)[:, None]
    return (u >= p + 512).astype(np.float32)


def _perm_even_odd(w, n_heads_w):
    # reorder each head's 64 columns: even indices first, then odd
    perm = np.concatenate([np.arange(0, HD, 2), np.arange(1, HD, 2)])
    wr = w.reshape(D, n_heads_w, HD)[:, :, perm]
    return np.ascontiguousarray(wr.reshape(D, n_heads_w * HD))


def _is_causal(mask, s):
    m = np.asarray(mask, dtype=np.float32).reshape(s, s)
    tri = np.tril(np.ones((s, s), dtype=bool))
    return bool(np.all(m[tri] == 0.0) and np.all(m[~tri] <= -1e8))


def _bf16(a):
    return np.ascontiguousarray(np.asarray(a, np.float32).astype(NPBF16))


def make_in_maps(x, mask, wq, wk, wv, wo, s=S):
    """Shard full inputs into 8 per-core input dicts."""
    causal = _is_causal(mask, s)
    c4, s4 = _rope_tables(s)
    wq_p = _perm_even_odd(np.asarray(wq, np.float32), N_HEADS)
    wk_p = _perm_even_odd(np.asarray(wk, np.float32), N_KV)
    wv = np.asarray(wv, np.float32)
    wo_b = _bf16(wo)
    ident = np.eye(128, dtype=np.float32).astype(NPBF16)
    psw = _pswap().astype(NPBF16)
    md = _mdiag().astype(NPBF16)
    xT = _bf16(np.asarray(x, np.float32).reshape(B * s, D).T)
    mT = None
    if not causal:
        mT = np.ascontiguousarray(np.asarray(mask, np.float32).reshape(s, s).T)

    in_maps = []
    for c in range(NCORES):
        wkv = np.concatenate([wk_p[:, c * HD:(c + 1) * HD],
                              wv[:, c * HD:(c + 1) * HD]], axis=1)
        im = {
            "xT": xT,
            "wq": _bf16(wq_p[:, c * QCOLS:(c + 1) * QCOLS]),
            "wkv": _bf16(wkv),
            "wo": wo_b,
            "c4": c4,
            "s4": s4,
            "ident": ident,
            "pswap": psw,
        }
        if causal:
            im["mdiag"] = md
        else:
            im["maskT"] = mT
        in_maps.append(im)
    return causal, in_maps


def assemble_output(per_core_outs, s=S):
    rows_h = s // NCORES
    y = np.empty((B, s, D), dtype=np.float32)
    for c in range(NCORES):
        o = np.asarray(per_core_outs[c], np.float32)
        for b in range(B):
            y[b, c * rows_h:(c + 1) * rows_h, :] = \
                o[b * rows_h:(b + 1) * rows_h]
    return y


_GRAPH_CACHE = {}


def get_graph(causal, s=S):
    key = (causal, s)
    if key not in _GRAPH_CACHE:
        _GRAPH_CACHE[key] = build_graph(causal, s)
    return _GRAPH_CACHE[key]


def kernel(**inputs):
    x = np.asarray(inputs["x"], np.float32)
    mask = inputs["mask"]
    s = x.shape[1]
    causal, in_maps = make_in_maps(x, mask, inputs["wq"], inputs["wk"],
                                   inputs["wv"], inputs["wo"], s=s)
    nc = get_graph(causal, s)
    res = bass_utils.run_bass_kernel_spmd(nc, in_maps, core_ids=list(range(NCORES)))
    return assemble_output([res.results[c]["out"] for c in range(NCORES)], s=s)
